# revision 1
# baseline (speedup 1.0000x reference)
"""Trainium2 Bass kernel for nn_BlockDense_89730456748629.

Block-diagonal dense layer + ReLU:
    out[b, g*H+h] = relu( sum_w inputs[b, g*WIN+w] * W[g*WIN+w, g*H+h] )
with G=32 groups, WIN=128, H=256, B=4096.

Sharding: group-parallel over 8 NeuronCores — core c owns groups
[4c, 4c+4). Each core gets the matching 512 input columns of `inputs`
(pre-transposed on host so the contraction dim lies on SBUF partitions)
plus its 4 diagonal W blocks, and produces the matching 1024 output
columns. No cross-core communication.

Per-core device pipeline:
  DMA xT group-row (2MB) -> PE matmul (lhsT = xT tile [128win,128b],
  rhs = W block [128win,256h], PSUM fp32) -> ReLU fused into the
  PSUM->SBUF copy (alternating VectorE / ScalarE) -> 1MB batched DMA out.
"""

import os
import time

import numpy as np

G, WIN, H, B = 32, 128, 256, 4096
NCORES = 8
GPC = G // NCORES            # groups per core
COLS_IN_PC = GPC * WIN       # 512 input columns per core
COLS_OUT_PC = GPC * H        # 1024 output columns per core
NB = B // 128                # 32 batch tiles of 128 rows

# dtype config: f32 | f32r | f16 | bf16 for inputs/matmul, f32 | f16 | bf16 out.
# Default f16 end-to-end: measured output error is dominated by the final
# f16 rounding (~5e-4 scale-relative max) while DMA bytes (the bottleneck)
# halve vs f32.
IN_DT = os.environ.get("KERNEL_IN_DT", "f16")
OUT_DT = os.environ.get("KERNEL_OUT_DT", "f16")
# batch tiles per out-DMA chunk (2-byte out: 16 -> 4MB chunks; 4-byte: 8)
CH = int(
    os.environ.get("KERNEL_CH", "16" if OUT_DT in ("f16", "bf16") else "8")
)
VERBOSE = os.environ.get("KERNEL_VERBOSE", "0") == "1"

_progs = {}


def _log(msg):
    if VERBOSE:
        print(f"[kernel] {msg}", flush=True)


def _np_dt(tag):
    if tag in ("f32", "f32r"):
        return np.dtype(np.float32)
    if tag == "f16":
        return np.dtype(np.float16)
    if tag == "bf16":
        import ml_dtypes

        return np.dtype(ml_dtypes.bfloat16)
    raise ValueError(tag)


def _mybir_dt(tag):
    from concourse import mybir

    return {
        "f32": mybir.dt.float32,
        "f32r": mybir.dt.float32r,
        "f16": mybir.dt.float16,
        "bf16": mybir.dt.bfloat16,
    }[tag]


def _build(in_tag, out_tag, repeat, loop_n=0):
    """Build the program. `repeat` = static unroll of the whole body;
    `loop_n` > 0 additionally wraps the unrolled body in a hardware
    For_i loop with that trip count (bench-only, for timing)."""
    from concourse import bacc, mybir, tile

    # bench-only ablations: comma-set of {noin,nomm,norelu,noout}
    ablate = set(filter(None, os.environ.get("KERNEL_ABLATE", "").split(",")))
    relu_eng = os.environ.get("KERNEL_RELU", "mix")  # mix | dve | act
    psw = int(os.environ.get("KERNEL_PSW", "512"))   # psum tile width (256|512)
    layout = os.environ.get("KERNEL_LAYOUT", "bchunk")  # bchunk | group
    # ring for input DMAs: "sync" = separate ring from out-DMAs (full
    # concurrency, HBM pays read/write turnaround), "act" = same ring as
    # out-DMAs (FIFO phases read bursts vs write bursts), "both" = alternate
    inring = os.environ.get("KERNEL_INRING", "sync")
    outring = os.environ.get("KERNEL_OUTRING", "act")  # act | both
    # phase=1: order in-DMA burst k+1 after the last out-DMA of k so HBM
    # sees alternating read/write bursts instead of mixed traffic
    phase = os.environ.get("KERNEL_PHASE", "0") == "1"

    in_dt = _mybir_dt(in_tag)
    out_dt = _mybir_dt(out_tag)

    nc = bacc.Bacc(
        "TRN2", target_bir_lowering=False, debug=False, num_devices=NCORES
    )
    xT = nc.declare_dram_parameter("xT", [COLS_IN_PC, B], in_dt, isOutput=False)
    Wb = nc.declare_dram_parameter("Wb", [WIN, COLS_OUT_PC], in_dt, isOutput=False)
    out = nc.declare_dram_parameter("out", [B, COLS_OUT_PC], out_dt, isOutput=True)

    out_v = out.rearrange("(nb p) w -> nb p w", p=128)  # (NB, 128, COLS_OUT_PC)

    in_sz = 2 if in_tag in ("f16", "bf16") else 4
    out_sz = 2 if out_tag in ("f16", "bf16") else 4
    if layout == "bchunk":
        # deep prefetch wins: 4 resident group tiles + 8 in flight ahead
        xbufs = 12 if in_sz == 2 else 6
        if out_sz == 2:
            obufs = 3 if CH >= 16 else 5
        else:
            obufs = 2
    else:
        xbufs, obufs = 2, 4
    xbufs = int(os.environ.get("KERNEL_XBUFS", xbufs))
    obufs = int(os.environ.get("KERNEL_OBUFS", obufs))

    with tile.TileContext(nc) as tc:
        with (
            tc.tile_pool(name="w", bufs=1) as wpool,
            tc.tile_pool(name="x", bufs=xbufs) as xpool,
            tc.tile_pool(name="o", bufs=obufs) as opool,
            tc.tile_pool(name="ps", bufs=8, space="PSUM") as pspool,
        ):
            wt = wpool.tile([WIN, COLS_OUT_PC], in_dt)
            nc.sync.dma_start(wt[:], Wb[:, :])

            relu_ct = [0]

            def relu(dst, src):
                pick = relu_eng
                if pick == "mix":
                    pick = "dve" if relu_ct[0] % 2 == 0 else "act"
                relu_ct[0] += 1
                if pick == "dve":
                    nc.vector.tensor_scalar_max(dst, src, 0.0)
                else:
                    nc.scalar.activation(
                        dst, src, mybir.ActivationFunctionType.Relu
                    )

            mm_per_ps = psw // H  # matmuls per psum tile (1 or 2)

            def body_group():
                """Group-outer: xt = one group row over all B; out-DMA
                writes H-wide column strips (512B runs at f16)."""
                for _rep in range(repeat):
                    for g in range(GPC):
                        xt = xpool.tile([WIN, B], in_dt)
                        if "noin" not in ablate:
                            nc.sync.dma_start(
                                xt[:], xT[g * WIN : (g + 1) * WIN, :]
                            )
                        for c in range(NB // CH):
                            ob = opool.tile([128, CH * H], out_dt)
                            for j2 in range(CH // mm_per_ps):
                                ps = pspool.tile([128, psw], mybir.dt.float32)
                                for h in range(mm_per_ps):
                                    bt = c * CH + j2 * mm_per_ps + h
                                    if "nomm" not in ablate:
                                        nc.tensor.matmul(
                                            ps[:, h * H : (h + 1) * H],
                                            xt[:, bt * 128 : (bt + 1) * 128],
                                            wt[:, g * H : (g + 1) * H],
                                            start=True,
                                            stop=True,
                                        )
                                if "norelu" not in ablate:
                                    relu(
                                        ob[:, j2 * psw : (j2 + 1) * psw],
                                        ps[:],
                                    )
                            if "noout" not in ablate:
                                dv = out_v[
                                    c * CH : (c + 1) * CH, :, g * H : (g + 1) * H
                                ].transpose([1, 0, 2])
                                # out-DMAs ride the ACT HWDGE ring so they
                                # overlap the input DMAs on the SP ring
                                # (FIFO per ring)
                                ob3 = ob[:].rearrange("p (j h) -> p j h", h=H)
                                nc.scalar.dma_start(dv, ob3)

            def body_bchunk():
                """B-chunk-outer: all 4 group tiles resident; out-DMA
                writes full COLS_OUT_PC-wide rows (2KB runs at f16)."""
                from concourse.tile import add_dep_helper

                in1 = os.environ.get("KERNEL_IN1", "0") == "1"
                prev_out = [None]
                for _rep in range(repeat):
                    if in1:
                        # one fused 4MB input DMA: xT shard rows (g, p) -> p g b
                        xt_all = xpool.tile([WIN, GPC, B], in_dt, tag="xt")
                        if "noin" not in ablate:
                            nc.sync.dma_start(
                                xt_all[:],
                                xT.rearrange("(g p) b -> p g b", p=WIN),
                            )
                        xts = [xt_all[:, g, :] for g in range(GPC)]
                    else:
                        xts = []
                    for g in range(GPC if not in1 else 0):
                        if inring == "both":
                            in_eng = nc.sync if g % 2 == 0 else nc.scalar
                        elif inring == "gpsimd":
                            in_eng = nc.gpsimd
                        else:
                            in_eng = nc.scalar if inring == "act" else nc.sync
                        xt = xpool.tile([WIN, B], in_dt, tag="xt")
                        if "noin" not in ablate:
                            if inring == "sync2":
                                # split each group read into two halves for
                                # more descriptors in flight
                                hb = B // 2
                                for s in range(2):
                                    di = nc.sync.dma_start(
                                        xt[:, s * hb : (s + 1) * hb],
                                        xT[
                                            g * WIN : (g + 1) * WIN,
                                            s * hb : (s + 1) * hb,
                                        ],
                                    )
                            else:
                                di = in_eng.dma_start(
                                    xt[:], xT[g * WIN : (g + 1) * WIN, :]
                                )
                            if phase and prev_out[0] is not None:
                                add_dep_helper(
                                    prev_out[0].ins,
                                    di.ins,
                                    True,
                                    "phase reads after writes",
                                )
                        xts.append(xt)
                    for c in range(NB // CH):
                        ob = opool.tile([128, CH, COLS_OUT_PC], out_dt)
                        if "norelu" in ablate and "noout" not in ablate:
                            # mark ob written so Tile allocates it (bench only)
                            nc.gpsimd.memset(ob[:, 0, 0:128], 0)
                        for g in range(GPC):
                            for j2 in range(CH // mm_per_ps):
                                ps = pspool.tile([128, psw], mybir.dt.float32)
                                for h in range(mm_per_ps):
                                    bt = c * CH + j2 * mm_per_ps + h
                                    if "nomm" not in ablate:
                                        nc.tensor.matmul(
                                            ps[:, h * H : (h + 1) * H],
                                            xts[g][:, bt * 128 : (bt + 1) * 128],
                                            wt[:, g * H : (g + 1) * H],
                                            start=True,
                                            stop=True,
                                        )
                                if "norelu" not in ablate:
                                    # psum [128, (j, h)] -> ob rows j2*m+h,
                                    # group-g column strip
                                    dst = ob[
                                        :,
                                        j2 * mm_per_ps : (j2 + 1) * mm_per_ps,
                                        g * H : (g + 1) * H,
                                    ]
                                    src = ps[:].rearrange(
                                        "p (j h) -> p j h", h=H
                                    )
                                    relu(dst, src)
                        if "noout" not in ablate:
                            dv = out_v[c * CH : (c + 1) * CH, :, :].transpose(
                                [1, 0, 2]
                            )
                            if outring == "both":
                                out_eng = nc.scalar if c % 2 == 0 else nc.sync
                            else:
                                out_eng = nc.scalar
                            do = out_eng.dma_start(dv, ob[:])
                            prev_out[0] = do

            body = body_bchunk if layout == "bchunk" else body_group

            if loop_n > 0:
                sreset = os.environ.get("KERNEL_SRESET", "0") == "1"
                with tc.For_i(0, loop_n, 1, staggered_reset=sreset):
                    body()
            else:
                body()
    nc.compile()
    return nc


def _make_runner(nc):
    """Cached jitted shard_map runner over 8 cores (modeled on
    concourse.bass2jax.run_bass_via_pjrt, but reusable across calls:
    the jitted fn and on-device zero output buffers are kept)."""
    import jax

    try:  # soften repeat first-call compiles across processes
        jax.config.update("jax_compilation_cache_dir", "/tmp/jax_bass_cache")
        jax.config.update("jax_persistent_cache_min_compile_time_secs", 1.0)
    except Exception:
        pass
    from jax.experimental.shard_map import shard_map
    from jax.sharding import Mesh, NamedSharding, PartitionSpec

    from concourse import mybir
    from concourse.bass2jax import (
        _bass_exec_p,
        install_neuronx_cc_hook,
        partition_id_tensor,
    )

    install_neuronx_cc_hook()

    partition_name = (
        nc.partition_id_tensor.name if nc.partition_id_tensor else None
    )
    in_names, out_names, out_avals = [], [], []
    for alloc in nc.m.functions[0].allocations:
        if not isinstance(alloc, mybir.MemoryLocationSet):
            continue
        name = alloc.memorylocations[0].name
        if alloc.kind == "ExternalInput":
            if name != partition_name:
                in_names.append(name)
        elif alloc.kind == "ExternalOutput":
            out_names.append(name)
            out_avals.append(
                jax.core.ShapedArray(
                    tuple(alloc.tensor_shape), mybir.dt.np(alloc.dtype)
                )
            )
    n_params = len(in_names)
    all_names = in_names + out_names
    if partition_name is not None:
        all_names = all_names + [partition_name]

    def _body(*args):
        operands = list(args)
        if partition_name is not None:
            operands.append(partition_id_tensor())
        outs = _bass_exec_p.bind(
            *operands,
            out_avals=tuple(out_avals),
            in_names=tuple(all_names),
            out_names=tuple(out_names),
            lowering_input_output_aliases=(),
            sim_require_finite=True,
            sim_require_nnan=True,
            nc=nc,
        )
        return tuple(outs)

    devices = jax.devices()[:NCORES]
    mesh = Mesh(np.asarray(devices), ("core",))
    nout = len(out_names)
    fn = jax.jit(
        shard_map(
            _body,
            mesh=mesh,
            in_specs=(PartitionSpec("core"),) * (n_params + nout),
            out_specs=(PartitionSpec("core"),) * nout,
            check_rep=False,
        ),
        keep_unused=True,
    )
    sharding = NamedSharding(mesh, PartitionSpec("core"))
    zeros = [
        jax.device_put(
            np.zeros((NCORES * a.shape[0], *a.shape[1:]), a.dtype), sharding
        )
        for a in out_avals
    ]
    return {
        "fn": fn,
        "in_names": in_names,
        "out_names": out_names,
        "out_avals": out_avals,
        "sharding": sharding,
        "zeros": zeros,
    }


def get_prog(repeat=1, loop_n=0):
    """Build (or fetch cached) compiled program + runner for the current
    dtype config and the given repeat-unroll / hw-loop factors."""
    key = (IN_DT, OUT_DT, repeat, loop_n)
    if key not in _progs:
        t0 = time.time()
        nc = _build(IN_DT, OUT_DT, repeat, loop_n)
        t1 = time.time()
        runner = _make_runner(nc)
        t2 = time.time()
        _log(
            f"built prog {key}: bass build+compile {t1 - t0:.1f}s, "
            f"runner setup {t2 - t1:.1f}s"
        )
        runner["nc"] = nc
        _progs[key] = runner
    return _progs[key]


def shard_inputs(inputs, W):
    """Host-side sharding: transpose x, extract diagonal W blocks, split
    per core, concat along axis 0 for shard_map consumption."""
    in_np = _np_dt(IN_DT)
    x = np.asarray(inputs, dtype=np.float32)
    Wf = np.asarray(W, dtype=np.float32)

    xT = np.ascontiguousarray(x.T)  # (G*WIN, B): row g*WIN+w = input col
    Wd = Wf.reshape(G, WIN, G, H)[np.arange(G), :, np.arange(G), :]  # (G,WIN,H)

    # concat over cores along axis 0 (shard_map splits axis 0 across mesh)
    xT_cat = xT.astype(in_np)  # already (NCORES*COLS_IN_PC, B) in core order
    Wb_cat = np.ascontiguousarray(
        Wd.reshape(NCORES, GPC, WIN, H)
        .transpose(0, 2, 1, 3)
        .reshape(NCORES * WIN, COLS_OUT_PC)
    ).astype(in_np)
    return {"xT": xT_cat, "Wb": Wb_cat}


def place_inputs(prog, cat_inputs):
    """device_put the sharded inputs once; reusable across run_prog calls."""
    import jax

    return [
        jax.device_put(cat_inputs[name], prog["sharding"])
        for name in prog["in_names"]
    ]


def run_prog(prog, cat_inputs=None, placed=None):
    """Run the program on 8 cores; returns output arrays (on device)."""
    import jax

    if placed is None:
        placed = place_inputs(prog, cat_inputs)
    outs = prog["fn"](*placed, *prog["zeros"])
    jax.block_until_ready(outs)
    return outs


def kernel(inputs, W):
    prog = get_prog(repeat=1)
    cat = shard_inputs(inputs, W)
    outs = run_prog(prog, cat)
    out_cat = np.asarray(outs[prog["out_names"].index("out")])
    # (NCORES*B, COLS_OUT_PC) -> (B, NCORES*COLS_OUT_PC)
    full = np.concatenate(
        [
            out_cat[c * B : (c + 1) * B].astype(np.float32)
            for c in range(NCORES)
        ],
        axis=1,
    )
    return full



# revision 22
# speedup vs baseline: 1.1881x; 1.1881x over previous
"""Trainium2 Bass kernel for nn_BlockDense_89730456748629.

Block-diagonal dense layer + ReLU:
    out[b, g*H+h] = relu( sum_w inputs[b, g*WIN+w] * W[g*WIN+w, g*H+h] )
with G=32 groups, WIN=128, H=256, B=4096.

Sharding: group-parallel over 8 NeuronCores — core c owns groups
[4c, 4c+4). Each core gets the matching 512 input columns of `inputs`
(pre-transposed on host so the contraction dim lies on SBUF partitions)
plus its 4 diagonal W blocks, and produces the matching 1024 output
columns. No cross-core communication.

Default layout "t2" (measured ~7% faster than the earlier "bchunk"):
  - W-block-half [128w,128h] is the stationary matmul operand; batch
    streams through at N=512 into fp32 PSUM banks -> PSUM lands h-major.
  - ReLU fused into the PSUM->SBUF f16 copy, alternating VectorE /
    ScalarE 1:1 (both ~0.6-0.7us per [128,512] tile; neither issues DMAs).
  - Output DRAM layout is partition-major [128, 8, B] per core (host
    decodes): per-partition runs are 16-64KB contiguous, so an 8MB write
    stream needs only ~512 descriptors (vs 4096 with 2KB runs) and the
    SP-ring sequencer issues just 4 dma_starts per iteration.
  - All DMAs ride the SP HWDGE ring; ACT/DVE do pure relu.

Measured rooflines on these cores (f16): writes ~345GB/s, reads
~390-425GB/s, mixed R/W ~310-330GB/s aggregate regardless of ring
split — the kernel is pinned at the mixed-traffic HBM wall (12MB/iter),
with PE (~16us) and relu (~21us) fully hidden.
"""

import os
import time

import numpy as np

G, WIN, H, B = 32, 128, 256, 4096
NCORES = 8
GPC = G // NCORES            # groups per core
COLS_IN_PC = GPC * WIN       # 512 input columns per core
COLS_OUT_PC = GPC * H        # 1024 output columns per core
NB = B // 128                # 32 batch tiles of 128 rows

# dtype config: f32 | f32r | f16 | bf16 for inputs/matmul, f32 | f16 | bf16 out.
# Default f16 end-to-end: measured output error is dominated by the final
# f16 rounding (~5e-4 scale-relative max) while DMA bytes (the bottleneck)
# halve vs f32.
IN_DT = os.environ.get("KERNEL_IN_DT", "f16")
OUT_DT = os.environ.get("KERNEL_OUT_DT", "f16")
# batch tiles per out-DMA chunk (2-byte out: 16 -> 4MB chunks; 4-byte: 8)
CH = int(
    os.environ.get("KERNEL_CH", "16" if OUT_DT in ("f16", "bf16") else "8")
)
VERBOSE = os.environ.get("KERNEL_VERBOSE", "0") == "1"
# single source of truth for the layout default (used by _build and kernel())
LAYOUT_DEFAULT = "t2"

_progs = {}


def _log(msg):
    if VERBOSE:
        print(f"[kernel] {msg}", flush=True)


def _np_dt(tag):
    if tag in ("f32", "f32r"):
        return np.dtype(np.float32)
    if tag == "f16":
        return np.dtype(np.float16)
    if tag == "bf16":
        import ml_dtypes

        return np.dtype(ml_dtypes.bfloat16)
    raise ValueError(tag)


def _mybir_dt(tag):
    from concourse import mybir

    return {
        "f32": mybir.dt.float32,
        "f32r": mybir.dt.float32r,
        "f16": mybir.dt.float16,
        "bf16": mybir.dt.bfloat16,
    }[tag]


def _build(in_tag, out_tag, repeat, loop_n=0):
    """Build the program. `repeat` = static unroll of the whole body;
    `loop_n` > 0 additionally wraps the unrolled body in a hardware
    For_i loop with that trip count (bench-only, for timing)."""
    from concourse import bacc, mybir, tile

    # bench-only ablations: comma-set of {noin,nomm,norelu,noout}
    ablate = set(filter(None, os.environ.get("KERNEL_ABLATE", "").split(",")))
    # relu engine split: dve | act | mix (alternate) | mixN (N dve : 1 act)
    relu_eng = os.environ.get("KERNEL_RELU", "mix")
    psw = int(os.environ.get("KERNEL_PSW", "512"))   # psum tile width (256|512)
    # psum dtype for tout layout: f32 | bf16 | f16 (16-bit doubles DVE rate)
    ps_tag = os.environ.get("KERNEL_PSDT", "f32")
    # bchunk | group | tout | t2. t2 measured ~7% faster than bchunk:
    # W-stationary matmuls, partition-major out layout (64KB runs, 32x
    # fewer descriptors), out-DMAs on the idle SP ring, ACT+DVE pure relu.
    layout = os.environ.get("KERNEL_LAYOUT", LAYOUT_DEFAULT)
    # ring for input DMAs: "sync" = separate ring from out-DMAs (full
    # concurrency, HBM pays read/write turnaround), "act" = same ring as
    # out-DMAs (FIFO phases read bursts vs write bursts), "both" = alternate
    inring = os.environ.get("KERNEL_INRING", "sync")
    outring = os.environ.get("KERNEL_OUTRING", "act")  # act | both
    # phase=1: order in-DMA burst k+1 after the last out-DMA of k so HBM
    # sees alternating read/write bursts instead of mixed traffic
    phase = os.environ.get("KERNEL_PHASE", "0") == "1"

    in_dt = _mybir_dt(in_tag)
    out_dt = _mybir_dt(out_tag)

    nc = bacc.Bacc(
        "TRN2", target_bir_lowering=False, debug=False, num_devices=NCORES
    )
    xT = nc.declare_dram_parameter("xT", [COLS_IN_PC, B], in_dt, isOutput=False)
    Wb = nc.declare_dram_parameter("Wb", [WIN, COLS_OUT_PC], in_dt, isOutput=False)
    if layout == "t2":
        # partition-major transposed output: out[p, k, b] = column k*128+p,
        # batch b. Per-partition DRAM runs are 8*B*2 = 64KB contiguous ->
        # 128 descriptors per 8MB. Host decodes.
        out = nc.declare_dram_parameter(
            "out", [128, GPC * (H // 128), B], out_dt, isOutput=True
        )
        out_v = None
    elif layout == "tout":
        # transposed output: row = output column (g*H + h), col = batch.
        # Host re-transposes; device DMA gets 8KB contiguous runs/partition.
        out = nc.declare_dram_parameter("out", [COLS_OUT_PC, B], out_dt, isOutput=True)
        out_v = None
    else:
        out = nc.declare_dram_parameter("out", [B, COLS_OUT_PC], out_dt, isOutput=True)
        out_v = out.rearrange("(nb p) w -> nb p w", p=128)  # (NB, 128, COLS_OUT_PC)

    in_sz = 2 if in_tag in ("f16", "bf16") else 4
    out_sz = 2 if out_tag in ("f16", "bf16") else 4
    if layout == "bchunk":
        # deep prefetch wins: 4 resident group tiles + 8 in flight ahead
        xbufs = 12 if in_sz == 2 else 6
        if out_sz == 2:
            obufs = 3 if CH >= 16 else 5
        else:
            obufs = 2
    elif layout == "tout":
        # xt and ob tiles are both 8KB/partition at f16
        xbufs = 8 if in_sz == 2 else 4
        obufs = 4 if out_sz == 2 else 2
    elif layout == "t2":
        ogrp = int(os.environ.get("KERNEL_OGRP", "2"))
        xbufs = 6 if in_sz == 2 else 4
        # ob tile = ogrp*8KB/partition; keep ~2 iterations in flight
        obufs = max(2, min(16 // ogrp, 6))
    else:
        xbufs, obufs = 2, 4
    xbufs = int(os.environ.get("KERNEL_XBUFS", xbufs))
    obufs = int(os.environ.get("KERNEL_OBUFS", obufs))

    with tile.TileContext(nc) as tc:
        with (
            tc.tile_pool(name="w", bufs=1) as wpool,
            tc.tile_pool(name="x", bufs=xbufs) as xpool,
            tc.tile_pool(name="o", bufs=obufs) as opool,
            tc.tile_pool(name="ps", bufs=8, space="PSUM") as pspool,
        ):
            wt = wpool.tile([WIN, COLS_OUT_PC], in_dt)
            nc.sync.dma_start(wt[:], Wb[:, :])

            relu_ct = [0]
            # mixN: N dve ops per 1 act op (cost model: roughly equal rates)
            mix_n = 1
            if relu_eng.startswith("mix") and len(relu_eng) > 3:
                mix_n = int(relu_eng[3:])
                assert mix_n <= 9, f"suspicious mix ratio {mix_n}"

            def relu(dst, src):
                pick = relu_eng
                if pick.startswith("mix"):
                    pick = "dve" if relu_ct[0] % (mix_n + 1) < mix_n else "act"
                relu_ct[0] += 1
                if pick == "dve":
                    nc.vector.tensor_scalar_max(dst, src, 0.0)
                else:
                    nc.scalar.activation(
                        dst, src, mybir.ActivationFunctionType.Relu
                    )

            mm_per_ps = psw // H  # matmuls per psum tile (1 or 2)

            def body_group():
                """Group-outer: xt = one group row over all B; out-DMA
                writes H-wide column strips (512B runs at f16)."""
                for _rep in range(repeat):
                    for g in range(GPC):
                        xt = xpool.tile([WIN, B], in_dt)
                        if "noin" not in ablate:
                            nc.sync.dma_start(
                                xt[:], xT[g * WIN : (g + 1) * WIN, :]
                            )
                        for c in range(NB // CH):
                            ob = opool.tile([128, CH * H], out_dt)
                            for j2 in range(CH // mm_per_ps):
                                ps = pspool.tile([128, psw], mybir.dt.float32)
                                for h in range(mm_per_ps):
                                    bt = c * CH + j2 * mm_per_ps + h
                                    if "nomm" not in ablate:
                                        nc.tensor.matmul(
                                            ps[:, h * H : (h + 1) * H],
                                            xt[:, bt * 128 : (bt + 1) * 128],
                                            wt[:, g * H : (g + 1) * H],
                                            start=True,
                                            stop=True,
                                        )
                                if "norelu" not in ablate:
                                    relu(
                                        ob[:, j2 * psw : (j2 + 1) * psw],
                                        ps[:],
                                    )
                            if "noout" not in ablate:
                                dv = out_v[
                                    c * CH : (c + 1) * CH, :, g * H : (g + 1) * H
                                ].transpose([1, 0, 2])
                                # out-DMAs ride the ACT HWDGE ring so they
                                # overlap the input DMAs on the SP ring
                                # (FIFO per ring)
                                ob3 = ob[:].rearrange("p (j h) -> p j h", h=H)
                                nc.scalar.dma_start(dv, ob3)

            def body_bchunk():
                """B-chunk-outer: all 4 group tiles resident; out-DMA
                writes full COLS_OUT_PC-wide rows (2KB runs at f16)."""
                from concourse.tile import add_dep_helper

                in1 = os.environ.get("KERNEL_IN1", "0") == "1"
                prev_out = [None]
                for _rep in range(repeat):
                    if in1:
                        # one fused 4MB input DMA: xT shard rows (g, p) -> p g b
                        xt_all = xpool.tile([WIN, GPC, B], in_dt, tag="xt")
                        if "noin" not in ablate:
                            nc.sync.dma_start(
                                xt_all[:],
                                xT.rearrange("(g p) b -> p g b", p=WIN),
                            )
                        xts = [xt_all[:, g, :] for g in range(GPC)]
                    else:
                        xts = []
                    for g in range(GPC if not in1 else 0):
                        if inring == "both":
                            in_eng = nc.sync if g % 2 == 0 else nc.scalar
                        elif inring == "gpsimd":
                            in_eng = nc.gpsimd
                        else:
                            in_eng = nc.scalar if inring == "act" else nc.sync
                        xt = xpool.tile([WIN, B], in_dt, tag="xt")
                        if "noin" not in ablate:
                            if inring == "sync2":
                                # split each group read into two halves for
                                # more descriptors in flight
                                hb = B // 2
                                for s in range(2):
                                    di = nc.sync.dma_start(
                                        xt[:, s * hb : (s + 1) * hb],
                                        xT[
                                            g * WIN : (g + 1) * WIN,
                                            s * hb : (s + 1) * hb,
                                        ],
                                    )
                            else:
                                di = in_eng.dma_start(
                                    xt[:], xT[g * WIN : (g + 1) * WIN, :]
                                )
                            if phase and prev_out[0] is not None:
                                add_dep_helper(
                                    prev_out[0].ins,
                                    di.ins,
                                    True,
                                    "phase reads after writes",
                                )
                        xts.append(xt)
                    for c in range(NB // CH):
                        ob = opool.tile([128, CH, COLS_OUT_PC], out_dt)
                        if "norelu" in ablate and "noout" not in ablate:
                            # mark ob written so Tile allocates it (bench only)
                            nc.gpsimd.memset(ob[:, 0, 0:128], 0)
                        for g in range(GPC):
                            for j2 in range(CH // mm_per_ps):
                                ps = pspool.tile([128, psw], mybir.dt.float32)
                                for h in range(mm_per_ps):
                                    bt = c * CH + j2 * mm_per_ps + h
                                    if "nomm" not in ablate:
                                        nc.tensor.matmul(
                                            ps[:, h * H : (h + 1) * H],
                                            xts[g][:, bt * 128 : (bt + 1) * 128],
                                            wt[:, g * H : (g + 1) * H],
                                            start=True,
                                            stop=True,
                                        )
                                if "norelu" not in ablate:
                                    # psum [128, (j, h)] -> ob rows j2*m+h,
                                    # group-g column strip
                                    dst = ob[
                                        :,
                                        j2 * mm_per_ps : (j2 + 1) * mm_per_ps,
                                        g * H : (g + 1) * H,
                                    ]
                                    src = ps[:].rearrange(
                                        "p (j h) -> p j h", h=H
                                    )
                                    relu(dst, src)
                        if "noout" not in ablate:
                            dv = out_v[c * CH : (c + 1) * CH, :, :].transpose(
                                [1, 0, 2]
                            )
                            if outring == "both":
                                out_eng = nc.scalar if c % 2 == 0 else nc.sync
                            elif outring == "sync":
                                out_eng = nc.sync
                            else:
                                out_eng = nc.scalar
                            do = out_eng.dma_start(dv, ob[:])
                            prev_out[0] = do

            def body_tout():
                """Transposed-output: W-block-half [128w,128h] is the
                stationary matmul operand, batch streams through at N=psw.
                PSUM lands [128h, b] so the out-DMA writes outT rows with
                8KB contiguous runs per partition. One 1MB out-DMA per
                (group, half), optionally striped across both HWDGE rings
                (KERNEL_OSPLIT=2)."""
                ps_dt = {
                    "f32": mybir.dt.float32,
                    "bf16": mybir.dt.bfloat16,
                    "f16": mybir.dt.float16,
                }[ps_tag]
                osplit = int(os.environ.get("KERNEL_OSPLIT", "1"))
                in1 = os.environ.get("KERNEL_IN1", "0") == "1"
                nhalf = H // 128  # 2 halves per group
                for _rep in range(repeat):
                    if in1:
                        # one fused 4MB input DMA: (g p) b -> p g b
                        xt_all = xpool.tile([WIN, GPC, B], in_dt, tag="xt")
                        if "noin" not in ablate:
                            nc.sync.dma_start(
                                xt_all[:],
                                xT.rearrange("(g p) b -> p g b", p=WIN),
                            )
                        xts = [xt_all[:, g, :] for g in range(GPC)]
                    else:
                        in_eng = nc.gpsimd if inring == "gpsimd" else nc.sync
                        xts = []
                        for g in range(GPC):
                            xt = xpool.tile([WIN, B], in_dt, tag="xt")
                            if "noin" not in ablate:
                                in_eng.dma_start(
                                    xt[:], xT[g * WIN : (g + 1) * WIN, :]
                                )
                            xts.append(xt)
                    for g in range(GPC):
                        for q in range(nhalf):
                            ob = opool.tile([128, B], out_dt, tag="ob")
                            if "norelu" in ablate and "noout" not in ablate:
                                nc.gpsimd.memset(ob[:, 0:128], 0)
                            wcol = g * H + q * 128
                            for j in range(B // psw):
                                ps = pspool.tile([128, psw], ps_dt)
                                if "nomm" not in ablate:
                                    nc.tensor.matmul(
                                        ps[:],
                                        wt[:, wcol : wcol + 128],
                                        xts[g][:, j * psw : (j + 1) * psw],
                                        start=True,
                                        stop=True,
                                    )
                                if "norelu" not in ablate:
                                    relu(ob[:, j * psw : (j + 1) * psw], ps[:])
                            if "noout" not in ablate:
                                r0 = (g * nhalf + q) * 128
                                k = g * nhalf + q
                                if osplit == 2:
                                    hb = B // 2
                                    for s in range(2):
                                        eng = nc.scalar if s == 0 else nc.sync
                                        eng.dma_start(
                                            out[
                                                r0 : r0 + 128,
                                                s * hb : (s + 1) * hb,
                                            ],
                                            ob[:, s * hb : (s + 1) * hb],
                                        )
                                else:
                                    if outring == "both":
                                        out_eng = (
                                            nc.scalar if k % 2 == 0 else nc.sync
                                        )
                                    elif outring == "sync":
                                        out_eng = nc.sync
                                    else:
                                        out_eng = nc.scalar
                                    out_eng.dma_start(
                                        out[r0 : r0 + 128, :], ob[:]
                                    )

            def body_t2():
                """Like tout but ob stages OGRP (group,half) strips in one
                [128, OGRP, B] tile and writes them with ONE dma_start into
                the [128, 8, B] DRAM layout (64KB runs/partition). Out-DMAs
                ride outring (default SP); ACT+DVE do pure relu."""
                ps_dt = {
                    "f32": mybir.dt.float32,
                    "bf16": mybir.dt.bfloat16,
                    "f16": mybir.dt.float16,
                }[ps_tag]
                ogrp = int(os.environ.get("KERNEL_OGRP", "2"))
                nhalf = H // 128
                nk = GPC * nhalf  # 8 column strips of 128
                for _rep in range(repeat):
                    in_eng = nc.gpsimd if inring == "gpsimd" else nc.sync
                    xts = []
                    for g in range(GPC):
                        xt = xpool.tile([WIN, B], in_dt, tag="xt")
                        if "noin" not in ablate:
                            in_eng.dma_start(
                                xt[:], xT[g * WIN : (g + 1) * WIN, :]
                            )
                        xts.append(xt)
                    for k0 in range(0, nk, ogrp):
                        ob = opool.tile([128, ogrp, B], out_dt, tag="ob")
                        if "norelu" in ablate and "noout" not in ablate:
                            nc.gpsimd.memset(ob[:, 0, 0:128], 0)
                        for kk in range(ogrp):
                            k = k0 + kk
                            g, q = divmod(k, nhalf)
                            wcol = g * H + q * 128
                            for j in range(B // psw):
                                ps = pspool.tile([128, psw], ps_dt)
                                if "nomm" not in ablate:
                                    nc.tensor.matmul(
                                        ps[:],
                                        wt[:, wcol : wcol + 128],
                                        xts[g][:, j * psw : (j + 1) * psw],
                                        start=True,
                                        stop=True,
                                    )
                                if "norelu" not in ablate:
                                    relu(
                                        ob[:, kk, j * psw : (j + 1) * psw],
                                        ps[:],
                                    )
                        if "noout" not in ablate:
                            if outring == "both":
                                out_eng = (
                                    nc.sync if (k0 // ogrp) % 2 == 0 else nc.scalar
                                )
                            elif outring == "act":
                                out_eng = nc.scalar
                            else:
                                out_eng = nc.sync
                            out_eng.dma_start(
                                out[:, k0 : k0 + ogrp, :], ob[:]
                            )

            body = {
                "bchunk": body_bchunk,
                "group": body_group,
                "tout": body_tout,
                "t2": body_t2,
            }[layout]

            if loop_n > 0:
                sreset = os.environ.get("KERNEL_SRESET", "0") == "1"
                with tc.For_i(0, loop_n, 1, staggered_reset=sreset):
                    body()
            else:
                body()
    nc.compile()
    return nc


def _make_runner(nc):
    """Cached jitted shard_map runner over 8 cores (modeled on
    concourse.bass2jax.run_bass_via_pjrt, but reusable across calls:
    the jitted fn and on-device zero output buffers are kept)."""
    import jax

    try:  # soften repeat first-call compiles across processes
        jax.config.update("jax_compilation_cache_dir", "/tmp/jax_bass_cache")
        jax.config.update("jax_persistent_cache_min_compile_time_secs", 1.0)
    except Exception:
        pass
    from jax.experimental.shard_map import shard_map
    from jax.sharding import Mesh, NamedSharding, PartitionSpec

    from concourse import mybir
    from concourse.bass2jax import (
        _bass_exec_p,
        install_neuronx_cc_hook,
        partition_id_tensor,
    )

    install_neuronx_cc_hook()

    partition_name = (
        nc.partition_id_tensor.name if nc.partition_id_tensor else None
    )
    in_names, out_names, out_avals = [], [], []
    for alloc in nc.m.functions[0].allocations:
        if not isinstance(alloc, mybir.MemoryLocationSet):
            continue
        name = alloc.memorylocations[0].name
        if alloc.kind == "ExternalInput":
            if name != partition_name:
                in_names.append(name)
        elif alloc.kind == "ExternalOutput":
            out_names.append(name)
            out_avals.append(
                jax.core.ShapedArray(
                    tuple(alloc.tensor_shape), mybir.dt.np(alloc.dtype)
                )
            )
    n_params = len(in_names)
    all_names = in_names + out_names
    if partition_name is not None:
        all_names = all_names + [partition_name]

    def _body(*args):
        operands = list(args)
        if partition_name is not None:
            operands.append(partition_id_tensor())
        outs = _bass_exec_p.bind(
            *operands,
            out_avals=tuple(out_avals),
            in_names=tuple(all_names),
            out_names=tuple(out_names),
            lowering_input_output_aliases=(),
            sim_require_finite=True,
            sim_require_nnan=True,
            nc=nc,
        )
        return tuple(outs)

    devices = jax.devices()[:NCORES]
    mesh = Mesh(np.asarray(devices), ("core",))
    nout = len(out_names)
    fn = jax.jit(
        shard_map(
            _body,
            mesh=mesh,
            in_specs=(PartitionSpec("core"),) * (n_params + nout),
            out_specs=(PartitionSpec("core"),) * nout,
            check_rep=False,
        ),
        keep_unused=True,
    )
    sharding = NamedSharding(mesh, PartitionSpec("core"))
    zeros = [
        jax.device_put(
            np.zeros((NCORES * a.shape[0], *a.shape[1:]), a.dtype), sharding
        )
        for a in out_avals
    ]
    return {
        "fn": fn,
        "in_names": in_names,
        "out_names": out_names,
        "out_avals": out_avals,
        "sharding": sharding,
        "zeros": zeros,
    }


def get_prog(repeat=1, loop_n=0):
    """Build (or fetch cached) compiled program + runner for the current
    dtype config and the given repeat-unroll / hw-loop factors."""
    key = (IN_DT, OUT_DT, repeat, loop_n)
    if key not in _progs:
        t0 = time.time()
        nc = _build(IN_DT, OUT_DT, repeat, loop_n)
        t1 = time.time()
        runner = _make_runner(nc)
        t2 = time.time()
        _log(
            f"built prog {key}: bass build+compile {t1 - t0:.1f}s, "
            f"runner setup {t2 - t1:.1f}s"
        )
        runner["nc"] = nc
        _progs[key] = runner
    return _progs[key]


def shard_inputs(inputs, W):
    """Host-side sharding: transpose x, extract diagonal W blocks, split
    per core, concat along axis 0 for shard_map consumption."""
    in_np = _np_dt(IN_DT)
    x = np.asarray(inputs, dtype=np.float32)
    Wf = np.asarray(W, dtype=np.float32)

    xT = np.ascontiguousarray(x.T)  # (G*WIN, B): row g*WIN+w = input col
    Wd = Wf.reshape(G, WIN, G, H)[np.arange(G), :, np.arange(G), :]  # (G,WIN,H)

    # concat over cores along axis 0 (shard_map splits axis 0 across mesh)
    xT_cat = xT.astype(in_np)  # already (NCORES*COLS_IN_PC, B) in core order
    Wb_cat = np.ascontiguousarray(
        Wd.reshape(NCORES, GPC, WIN, H)
        .transpose(0, 2, 1, 3)
        .reshape(NCORES * WIN, COLS_OUT_PC)
    ).astype(in_np)
    return {"xT": xT_cat, "Wb": Wb_cat}


def place_inputs(prog, cat_inputs):
    """device_put the sharded inputs once; reusable across run_prog calls."""
    import jax

    return [
        jax.device_put(cat_inputs[name], prog["sharding"])
        for name in prog["in_names"]
    ]


def run_prog(prog, cat_inputs=None, placed=None):
    """Run the program on 8 cores; returns output arrays (on device)."""
    import jax

    if placed is None:
        placed = place_inputs(prog, cat_inputs)
    outs = prog["fn"](*placed, *prog["zeros"])
    jax.block_until_ready(outs)
    return outs


def kernel(inputs, W):
    prog = get_prog(repeat=1)
    cat = shard_inputs(inputs, W)
    outs = run_prog(prog, cat)
    out_cat = np.asarray(outs[prog["out_names"].index("out")])
    if os.environ.get("KERNEL_LAYOUT", LAYOUT_DEFAULT) == "t2":
        # (NCORES*128, 8, B): [c*128+p, k, b] -> full[b, c*1024 + k*128 + p]
        nk = COLS_OUT_PC // 128
        full = (
            out_cat.reshape(NCORES, 128, nk, B)
            .transpose(3, 0, 2, 1)
            .reshape(B, NCORES * COLS_OUT_PC)
            .astype(np.float32)
        )
    elif os.environ.get("KERNEL_LAYOUT", LAYOUT_DEFAULT) == "tout":
        # (NCORES*COLS_OUT_PC, B) -> (B, NCORES*COLS_OUT_PC)
        full = (
            out_cat.reshape(NCORES, COLS_OUT_PC, B)
            .transpose(2, 0, 1)
            .reshape(B, NCORES * COLS_OUT_PC)
            .astype(np.float32)
        )
    else:
        # (NCORES*B, COLS_OUT_PC) -> (B, NCORES*COLS_OUT_PC)
        full = np.concatenate(
            [
                out_cat[c * B : (c + 1) * B].astype(np.float32)
                for c in range(NCORES)
            ],
            axis=1,
        )
    return full



# revision 27
# speedup vs baseline: 1.2255x; 1.0315x over previous
"""Trainium2 Bass kernel for nn_BlockDense_89730456748629.

Block-diagonal dense layer + ReLU:
    out[b, g*H+h] = relu( sum_w inputs[b, g*WIN+w] * W[g*WIN+w, g*H+h] )
with G=32 groups, WIN=128, H=256, B=4096.

Sharding: group-parallel over 8 NeuronCores — core c owns groups
[4c, 4c+4). Each core gets the matching 512 input columns of `inputs`
(pre-transposed on host so the contraction dim lies on SBUF partitions)
plus its 4 diagonal W blocks, and produces the matching 1024 output
columns. No cross-core communication.

Default layout "t2" (measured ~7% faster than the earlier "bchunk"):
  - W-block-half [128w,128h] is the stationary matmul operand; batch
    streams through at N=512 into fp32 PSUM banks -> PSUM lands h-major.
  - ReLU fused into the PSUM->SBUF f16 copy, alternating VectorE /
    ScalarE 1:1 (both ~0.6-0.7us per [128,512] tile; neither issues DMAs).
  - Output DRAM layout is partition-major [128, 8, B] per core (host
    decodes): per-partition runs are 16-64KB contiguous, so an 8MB write
    stream needs only ~512 descriptors (vs 4096 with 2KB runs) and the
    SP-ring sequencer issues just 4 dma_starts per iteration.
  - All DMAs ride the SP HWDGE ring; ACT/DVE do pure relu.
  - Reads for two iterations are fetched in one burst (KERNEL_INBURST=2),
    halving HBM read/write direction switches (~1us/iter measured).

Measured rooflines on these cores (f16): writes ~345GB/s, reads
~390-425GB/s, mixed R/W ~310-330GB/s aggregate regardless of ring
split — the kernel is pinned at the mixed-traffic HBM wall (12MB/iter),
with PE (~16us) and relu (~21us) fully hidden.
"""

import os
import time

import numpy as np

G, WIN, H, B = 32, 128, 256, 4096
NCORES = 8
GPC = G // NCORES            # groups per core
COLS_IN_PC = GPC * WIN       # 512 input columns per core
COLS_OUT_PC = GPC * H        # 1024 output columns per core
NB = B // 128                # 32 batch tiles of 128 rows

# dtype config: f32 | f32r | f16 | bf16 for inputs/matmul, f32 | f16 | bf16 out.
# Default f16 end-to-end: measured output error is dominated by the final
# f16 rounding (~5e-4 scale-relative max) while DMA bytes (the bottleneck)
# halve vs f32.
IN_DT = os.environ.get("KERNEL_IN_DT", "f16")
OUT_DT = os.environ.get("KERNEL_OUT_DT", "f16")
# batch tiles per out-DMA chunk (2-byte out: 16 -> 4MB chunks; 4-byte: 8)
CH = int(
    os.environ.get("KERNEL_CH", "16" if OUT_DT in ("f16", "bf16") else "8")
)
VERBOSE = os.environ.get("KERNEL_VERBOSE", "0") == "1"
# single source of truth for the layout default (used by _build and kernel())
LAYOUT_DEFAULT = "t2"

_progs = {}


def _log(msg):
    if VERBOSE:
        print(f"[kernel] {msg}", flush=True)


def _np_dt(tag):
    if tag in ("f32", "f32r"):
        return np.dtype(np.float32)
    if tag == "f16":
        return np.dtype(np.float16)
    if tag == "bf16":
        import ml_dtypes

        return np.dtype(ml_dtypes.bfloat16)
    raise ValueError(tag)


def _mybir_dt(tag):
    from concourse import mybir

    return {
        "f32": mybir.dt.float32,
        "f32r": mybir.dt.float32r,
        "f16": mybir.dt.float16,
        "bf16": mybir.dt.bfloat16,
    }[tag]


def _build(in_tag, out_tag, repeat, loop_n=0):
    """Build the program. `repeat` = static unroll of the whole body;
    `loop_n` > 0 additionally wraps the unrolled body in a hardware
    For_i loop with that trip count (bench-only, for timing)."""
    from concourse import bacc, mybir, tile

    # bench-only ablations: comma-set of {noin,nomm,norelu,noout}
    ablate = set(filter(None, os.environ.get("KERNEL_ABLATE", "").split(",")))
    # relu engine split: dve | act | mix (alternate) | mixN (N dve : 1 act)
    relu_eng = os.environ.get("KERNEL_RELU", "mix")
    psw = int(os.environ.get("KERNEL_PSW", "512"))   # psum tile width (256|512)
    # psum dtype for tout layout: f32 | bf16 | f16 (16-bit doubles DVE rate)
    ps_tag = os.environ.get("KERNEL_PSDT", "f32")
    # bchunk | group | tout | t2. t2 measured ~7% faster than bchunk:
    # W-stationary matmuls, partition-major out layout (64KB runs, 32x
    # fewer descriptors), out-DMAs on the idle SP ring, ACT+DVE pure relu.
    layout = os.environ.get("KERNEL_LAYOUT", LAYOUT_DEFAULT)
    # ring for input DMAs: "sync" = separate ring from out-DMAs (full
    # concurrency, HBM pays read/write turnaround), "act" = same ring as
    # out-DMAs (FIFO phases read bursts vs write bursts), "both" = alternate
    inring = os.environ.get("KERNEL_INRING", "sync")
    outring = os.environ.get("KERNEL_OUTRING", "act")  # act | both
    # phase=1: order in-DMA burst k+1 after the last out-DMA of k so HBM
    # sees alternating read/write bursts instead of mixed traffic
    phase = os.environ.get("KERNEL_PHASE", "0") == "1"

    in_dt = _mybir_dt(in_tag)
    out_dt = _mybir_dt(out_tag)

    nc = bacc.Bacc(
        "TRN2", target_bir_lowering=False, debug=False, num_devices=NCORES
    )
    xT = nc.declare_dram_parameter("xT", [COLS_IN_PC, B], in_dt, isOutput=False)
    Wb = nc.declare_dram_parameter("Wb", [WIN, COLS_OUT_PC], in_dt, isOutput=False)
    if layout == "t2":
        # partition-major transposed output: out[p, k, b] = column k*128+p,
        # batch b. Per-partition DRAM runs are 8*B*2 = 64KB contiguous ->
        # 128 descriptors per 8MB. Host decodes.
        out = nc.declare_dram_parameter(
            "out", [128, GPC * (H // 128), B], out_dt, isOutput=True
        )
        out_v = None
    elif layout == "tout":
        # transposed output: row = output column (g*H + h), col = batch.
        # Host re-transposes; device DMA gets 8KB contiguous runs/partition.
        out = nc.declare_dram_parameter("out", [COLS_OUT_PC, B], out_dt, isOutput=True)
        out_v = None
    else:
        out = nc.declare_dram_parameter("out", [B, COLS_OUT_PC], out_dt, isOutput=True)
        out_v = out.rearrange("(nb p) w -> nb p w", p=128)  # (NB, 128, COLS_OUT_PC)

    in_sz = 2 if in_tag in ("f16", "bf16") else 4
    out_sz = 2 if out_tag in ("f16", "bf16") else 4
    if layout == "bchunk":
        # deep prefetch wins: 4 resident group tiles + 8 in flight ahead
        xbufs = 12 if in_sz == 2 else 6
        if out_sz == 2:
            obufs = 3 if CH >= 16 else 5
        else:
            obufs = 2
    elif layout == "tout":
        # xt and ob tiles are both 8KB/partition at f16
        xbufs = 8 if in_sz == 2 else 4
        obufs = 4 if out_sz == 2 else 2
    elif layout == "t2":
        ogrp = int(os.environ.get("KERNEL_OGRP", "2"))
        inb = int(os.environ.get("KERNEL_INBURST", "2"))
        # 4*inburst resident xt tiles + prefetch margin (8KB/partition each)
        xbufs = (6 if in_sz == 2 else 4) + 4 * (inb - 1)
        # ob tile = ogrp*8KB/partition; keep ~2 iterations in flight
        obufs = max(2, min(16 // ogrp, 6 if inb == 1 else 4))
    else:
        xbufs, obufs = 2, 4
    xbufs = int(os.environ.get("KERNEL_XBUFS", xbufs))
    obufs = int(os.environ.get("KERNEL_OBUFS", obufs))

    with tile.TileContext(nc) as tc:
        with (
            tc.tile_pool(name="w", bufs=1) as wpool,
            tc.tile_pool(name="x", bufs=xbufs) as xpool,
            tc.tile_pool(name="o", bufs=obufs) as opool,
            tc.tile_pool(name="ps", bufs=8, space="PSUM") as pspool,
        ):
            wt = wpool.tile([WIN, COLS_OUT_PC], in_dt)
            nc.sync.dma_start(wt[:], Wb[:, :])

            relu_ct = [0]
            # mixN: N dve ops per 1 act op (cost model: roughly equal rates)
            mix_n = 1
            if relu_eng.startswith("mix") and len(relu_eng) > 3:
                mix_n = int(relu_eng[3:])
                assert mix_n <= 9, f"suspicious mix ratio {mix_n}"

            def relu(dst, src):
                pick = relu_eng
                if pick.startswith("mix"):
                    pick = "dve" if relu_ct[0] % (mix_n + 1) < mix_n else "act"
                relu_ct[0] += 1
                if pick == "dve":
                    nc.vector.tensor_scalar_max(dst, src, 0.0)
                else:
                    nc.scalar.activation(
                        dst, src, mybir.ActivationFunctionType.Relu
                    )

            mm_per_ps = psw // H  # matmuls per psum tile (1 or 2)

            def body_group():
                """Group-outer: xt = one group row over all B; out-DMA
                writes H-wide column strips (512B runs at f16)."""
                for _rep in range(repeat):
                    for g in range(GPC):
                        xt = xpool.tile([WIN, B], in_dt)
                        if "noin" not in ablate:
                            nc.sync.dma_start(
                                xt[:], xT[g * WIN : (g + 1) * WIN, :]
                            )
                        for c in range(NB // CH):
                            ob = opool.tile([128, CH * H], out_dt)
                            for j2 in range(CH // mm_per_ps):
                                ps = pspool.tile([128, psw], mybir.dt.float32)
                                for h in range(mm_per_ps):
                                    bt = c * CH + j2 * mm_per_ps + h
                                    if "nomm" not in ablate:
                                        nc.tensor.matmul(
                                            ps[:, h * H : (h + 1) * H],
                                            xt[:, bt * 128 : (bt + 1) * 128],
                                            wt[:, g * H : (g + 1) * H],
                                            start=True,
                                            stop=True,
                                        )
                                if "norelu" not in ablate:
                                    relu(
                                        ob[:, j2 * psw : (j2 + 1) * psw],
                                        ps[:],
                                    )
                            if "noout" not in ablate:
                                dv = out_v[
                                    c * CH : (c + 1) * CH, :, g * H : (g + 1) * H
                                ].transpose([1, 0, 2])
                                # out-DMAs ride the ACT HWDGE ring so they
                                # overlap the input DMAs on the SP ring
                                # (FIFO per ring)
                                ob3 = ob[:].rearrange("p (j h) -> p j h", h=H)
                                nc.scalar.dma_start(dv, ob3)

            def body_bchunk():
                """B-chunk-outer: all 4 group tiles resident; out-DMA
                writes full COLS_OUT_PC-wide rows (2KB runs at f16)."""
                from concourse.tile import add_dep_helper

                in1 = os.environ.get("KERNEL_IN1", "0") == "1"
                prev_out = [None]
                for _rep in range(repeat):
                    if in1:
                        # one fused 4MB input DMA: xT shard rows (g, p) -> p g b
                        xt_all = xpool.tile([WIN, GPC, B], in_dt, tag="xt")
                        if "noin" not in ablate:
                            nc.sync.dma_start(
                                xt_all[:],
                                xT.rearrange("(g p) b -> p g b", p=WIN),
                            )
                        xts = [xt_all[:, g, :] for g in range(GPC)]
                    else:
                        xts = []
                    for g in range(GPC if not in1 else 0):
                        if inring == "both":
                            in_eng = nc.sync if g % 2 == 0 else nc.scalar
                        elif inring == "gpsimd":
                            in_eng = nc.gpsimd
                        else:
                            in_eng = nc.scalar if inring == "act" else nc.sync
                        xt = xpool.tile([WIN, B], in_dt, tag="xt")
                        if "noin" not in ablate:
                            if inring == "sync2":
                                # split each group read into two halves for
                                # more descriptors in flight
                                hb = B // 2
                                for s in range(2):
                                    di = nc.sync.dma_start(
                                        xt[:, s * hb : (s + 1) * hb],
                                        xT[
                                            g * WIN : (g + 1) * WIN,
                                            s * hb : (s + 1) * hb,
                                        ],
                                    )
                            else:
                                di = in_eng.dma_start(
                                    xt[:], xT[g * WIN : (g + 1) * WIN, :]
                                )
                            if phase and prev_out[0] is not None:
                                add_dep_helper(
                                    prev_out[0].ins,
                                    di.ins,
                                    True,
                                    "phase reads after writes",
                                )
                        xts.append(xt)
                    for c in range(NB // CH):
                        ob = opool.tile([128, CH, COLS_OUT_PC], out_dt)
                        if "norelu" in ablate and "noout" not in ablate:
                            # mark ob written so Tile allocates it (bench only)
                            nc.gpsimd.memset(ob[:, 0, 0:128], 0)
                        for g in range(GPC):
                            for j2 in range(CH // mm_per_ps):
                                ps = pspool.tile([128, psw], mybir.dt.float32)
                                for h in range(mm_per_ps):
                                    bt = c * CH + j2 * mm_per_ps + h
                                    if "nomm" not in ablate:
                                        nc.tensor.matmul(
                                            ps[:, h * H : (h + 1) * H],
                                            xts[g][:, bt * 128 : (bt + 1) * 128],
                                            wt[:, g * H : (g + 1) * H],
                                            start=True,
                                            stop=True,
                                        )
                                if "norelu" not in ablate:
                                    # psum [128, (j, h)] -> ob rows j2*m+h,
                                    # group-g column strip
                                    dst = ob[
                                        :,
                                        j2 * mm_per_ps : (j2 + 1) * mm_per_ps,
                                        g * H : (g + 1) * H,
                                    ]
                                    src = ps[:].rearrange(
                                        "p (j h) -> p j h", h=H
                                    )
                                    relu(dst, src)
                        if "noout" not in ablate:
                            dv = out_v[c * CH : (c + 1) * CH, :, :].transpose(
                                [1, 0, 2]
                            )
                            if outring == "both":
                                out_eng = nc.scalar if c % 2 == 0 else nc.sync
                            elif outring == "sync":
                                out_eng = nc.sync
                            else:
                                out_eng = nc.scalar
                            do = out_eng.dma_start(dv, ob[:])
                            prev_out[0] = do

            def body_tout():
                """Transposed-output: W-block-half [128w,128h] is the
                stationary matmul operand, batch streams through at N=psw.
                PSUM lands [128h, b] so the out-DMA writes outT rows with
                8KB contiguous runs per partition. One 1MB out-DMA per
                (group, half), optionally striped across both HWDGE rings
                (KERNEL_OSPLIT=2)."""
                ps_dt = {
                    "f32": mybir.dt.float32,
                    "bf16": mybir.dt.bfloat16,
                    "f16": mybir.dt.float16,
                }[ps_tag]
                osplit = int(os.environ.get("KERNEL_OSPLIT", "1"))
                in1 = os.environ.get("KERNEL_IN1", "0") == "1"
                nhalf = H // 128  # 2 halves per group
                for _rep in range(repeat):
                    if in1:
                        # one fused 4MB input DMA: (g p) b -> p g b
                        xt_all = xpool.tile([WIN, GPC, B], in_dt, tag="xt")
                        if "noin" not in ablate:
                            nc.sync.dma_start(
                                xt_all[:],
                                xT.rearrange("(g p) b -> p g b", p=WIN),
                            )
                        xts = [xt_all[:, g, :] for g in range(GPC)]
                    else:
                        in_eng = nc.gpsimd if inring == "gpsimd" else nc.sync
                        xts = []
                        for g in range(GPC):
                            xt = xpool.tile([WIN, B], in_dt, tag="xt")
                            if "noin" not in ablate:
                                in_eng.dma_start(
                                    xt[:], xT[g * WIN : (g + 1) * WIN, :]
                                )
                            xts.append(xt)
                    for g in range(GPC):
                        for q in range(nhalf):
                            ob = opool.tile([128, B], out_dt, tag="ob")
                            if "norelu" in ablate and "noout" not in ablate:
                                nc.gpsimd.memset(ob[:, 0:128], 0)
                            wcol = g * H + q * 128
                            for j in range(B // psw):
                                ps = pspool.tile([128, psw], ps_dt)
                                if "nomm" not in ablate:
                                    nc.tensor.matmul(
                                        ps[:],
                                        wt[:, wcol : wcol + 128],
                                        xts[g][:, j * psw : (j + 1) * psw],
                                        start=True,
                                        stop=True,
                                    )
                                if "norelu" not in ablate:
                                    relu(ob[:, j * psw : (j + 1) * psw], ps[:])
                            if "noout" not in ablate:
                                r0 = (g * nhalf + q) * 128
                                k = g * nhalf + q
                                if osplit == 2:
                                    hb = B // 2
                                    for s in range(2):
                                        eng = nc.scalar if s == 0 else nc.sync
                                        eng.dma_start(
                                            out[
                                                r0 : r0 + 128,
                                                s * hb : (s + 1) * hb,
                                            ],
                                            ob[:, s * hb : (s + 1) * hb],
                                        )
                                else:
                                    if outring == "both":
                                        out_eng = (
                                            nc.scalar if k % 2 == 0 else nc.sync
                                        )
                                    elif outring == "sync":
                                        out_eng = nc.sync
                                    else:
                                        out_eng = nc.scalar
                                    out_eng.dma_start(
                                        out[r0 : r0 + 128, :], ob[:]
                                    )

            def body_t2():
                """Like tout but ob stages OGRP (group,half) strips in one
                [128, OGRP, B] tile and writes them with ONE dma_start into
                the [128, 8, B] DRAM layout (64KB runs/partition). Out-DMAs
                ride outring (default SP); ACT+DVE do pure relu."""
                ps_dt = {
                    "f32": mybir.dt.float32,
                    "bf16": mybir.dt.bfloat16,
                    "f16": mybir.dt.float16,
                }[ps_tag]
                ogrp = int(os.environ.get("KERNEL_OGRP", "2"))
                # read-burst width in iterations: fetching 2 iterations of x
                # per burst halves HBM read/write direction switches
                inburst = int(os.environ.get("KERNEL_INBURST", "2"))
                nhalf = H // 128
                nk = GPC * nhalf  # 8 column strips of 128
                in_eng = nc.gpsimd if inring == "gpsimd" else nc.sync
                xts_pending = {}
                for _rep in range(repeat):
                    if _rep % inburst == 0:
                        for r2 in range(_rep, min(_rep + inburst, repeat)):
                            for g in range(GPC):
                                xt = xpool.tile([WIN, B], in_dt, tag="xt")
                                if "noin" not in ablate:
                                    in_eng.dma_start(
                                        xt[:], xT[g * WIN : (g + 1) * WIN, :]
                                    )
                                xts_pending[(r2, g)] = xt
                    xts = [xts_pending.pop((_rep, g)) for g in range(GPC)]
                    for k0 in range(0, nk, ogrp):
                        ob = opool.tile([128, ogrp, B], out_dt, tag="ob")
                        if "norelu" in ablate and "noout" not in ablate:
                            nc.gpsimd.memset(ob[:, 0, 0:128], 0)
                        for kk in range(ogrp):
                            k = k0 + kk
                            g, q = divmod(k, nhalf)
                            wcol = g * H + q * 128
                            for j in range(B // psw):
                                ps = pspool.tile([128, psw], ps_dt)
                                if "nomm" not in ablate:
                                    nc.tensor.matmul(
                                        ps[:],
                                        wt[:, wcol : wcol + 128],
                                        xts[g][:, j * psw : (j + 1) * psw],
                                        start=True,
                                        stop=True,
                                    )
                                if "norelu" not in ablate:
                                    relu(
                                        ob[:, kk, j * psw : (j + 1) * psw],
                                        ps[:],
                                    )
                        if "noout" not in ablate:
                            if outring == "both":
                                out_eng = (
                                    nc.sync if (k0 // ogrp) % 2 == 0 else nc.scalar
                                )
                            elif outring == "act":
                                out_eng = nc.scalar
                            else:
                                out_eng = nc.sync
                            out_eng.dma_start(
                                out[:, k0 : k0 + ogrp, :], ob[:]
                            )

            body = {
                "bchunk": body_bchunk,
                "group": body_group,
                "tout": body_tout,
                "t2": body_t2,
            }[layout]

            if loop_n > 0:
                sreset = os.environ.get("KERNEL_SRESET", "0") == "1"
                with tc.For_i(0, loop_n, 1, staggered_reset=sreset):
                    body()
            else:
                body()
    nc.compile()
    return nc


def _make_runner(nc):
    """Cached jitted shard_map runner over 8 cores (modeled on
    concourse.bass2jax.run_bass_via_pjrt, but reusable across calls:
    the jitted fn and on-device zero output buffers are kept)."""
    import jax

    try:  # soften repeat first-call compiles across processes
        jax.config.update("jax_compilation_cache_dir", "/tmp/jax_bass_cache")
        jax.config.update("jax_persistent_cache_min_compile_time_secs", 1.0)
    except Exception:
        pass
    from jax.experimental.shard_map import shard_map
    from jax.sharding import Mesh, NamedSharding, PartitionSpec

    from concourse import mybir
    from concourse.bass2jax import (
        _bass_exec_p,
        install_neuronx_cc_hook,
        partition_id_tensor,
    )

    install_neuronx_cc_hook()

    partition_name = (
        nc.partition_id_tensor.name if nc.partition_id_tensor else None
    )
    in_names, out_names, out_avals = [], [], []
    for alloc in nc.m.functions[0].allocations:
        if not isinstance(alloc, mybir.MemoryLocationSet):
            continue
        name = alloc.memorylocations[0].name
        if alloc.kind == "ExternalInput":
            if name != partition_name:
                in_names.append(name)
        elif alloc.kind == "ExternalOutput":
            out_names.append(name)
            out_avals.append(
                jax.core.ShapedArray(
                    tuple(alloc.tensor_shape), mybir.dt.np(alloc.dtype)
                )
            )
    n_params = len(in_names)
    all_names = in_names + out_names
    if partition_name is not None:
        all_names = all_names + [partition_name]

    def _body(*args):
        operands = list(args)
        if partition_name is not None:
            operands.append(partition_id_tensor())
        outs = _bass_exec_p.bind(
            *operands,
            out_avals=tuple(out_avals),
            in_names=tuple(all_names),
            out_names=tuple(out_names),
            lowering_input_output_aliases=(),
            sim_require_finite=True,
            sim_require_nnan=True,
            nc=nc,
        )
        return tuple(outs)

    devices = jax.devices()[:NCORES]
    mesh = Mesh(np.asarray(devices), ("core",))
    nout = len(out_names)
    fn = jax.jit(
        shard_map(
            _body,
            mesh=mesh,
            in_specs=(PartitionSpec("core"),) * (n_params + nout),
            out_specs=(PartitionSpec("core"),) * nout,
            check_rep=False,
        ),
        keep_unused=True,
    )
    sharding = NamedSharding(mesh, PartitionSpec("core"))
    zeros = [
        jax.device_put(
            np.zeros((NCORES * a.shape[0], *a.shape[1:]), a.dtype), sharding
        )
        for a in out_avals
    ]
    return {
        "fn": fn,
        "in_names": in_names,
        "out_names": out_names,
        "out_avals": out_avals,
        "sharding": sharding,
        "zeros": zeros,
    }


def get_prog(repeat=1, loop_n=0):
    """Build (or fetch cached) compiled program + runner for the current
    dtype config and the given repeat-unroll / hw-loop factors."""
    key = (IN_DT, OUT_DT, repeat, loop_n)
    if key not in _progs:
        t0 = time.time()
        nc = _build(IN_DT, OUT_DT, repeat, loop_n)
        t1 = time.time()
        runner = _make_runner(nc)
        t2 = time.time()
        _log(
            f"built prog {key}: bass build+compile {t1 - t0:.1f}s, "
            f"runner setup {t2 - t1:.1f}s"
        )
        runner["nc"] = nc
        _progs[key] = runner
    return _progs[key]


def shard_inputs(inputs, W):
    """Host-side sharding: transpose x, extract diagonal W blocks, split
    per core, concat along axis 0 for shard_map consumption."""
    in_np = _np_dt(IN_DT)
    x = np.asarray(inputs, dtype=np.float32)
    Wf = np.asarray(W, dtype=np.float32)

    xT = np.ascontiguousarray(x.T)  # (G*WIN, B): row g*WIN+w = input col
    Wd = Wf.reshape(G, WIN, G, H)[np.arange(G), :, np.arange(G), :]  # (G,WIN,H)

    # concat over cores along axis 0 (shard_map splits axis 0 across mesh)
    xT_cat = xT.astype(in_np)  # already (NCORES*COLS_IN_PC, B) in core order
    Wb_cat = np.ascontiguousarray(
        Wd.reshape(NCORES, GPC, WIN, H)
        .transpose(0, 2, 1, 3)
        .reshape(NCORES * WIN, COLS_OUT_PC)
    ).astype(in_np)
    return {"xT": xT_cat, "Wb": Wb_cat}


def place_inputs(prog, cat_inputs):
    """device_put the sharded inputs once; reusable across run_prog calls."""
    import jax

    return [
        jax.device_put(cat_inputs[name], prog["sharding"])
        for name in prog["in_names"]
    ]


def run_prog(prog, cat_inputs=None, placed=None):
    """Run the program on 8 cores; returns output arrays (on device)."""
    import jax

    if placed is None:
        placed = place_inputs(prog, cat_inputs)
    outs = prog["fn"](*placed, *prog["zeros"])
    jax.block_until_ready(outs)
    return outs


def kernel(inputs, W):
    prog = get_prog(repeat=1)
    cat = shard_inputs(inputs, W)
    outs = run_prog(prog, cat)
    out_cat = np.asarray(outs[prog["out_names"].index("out")])
    if os.environ.get("KERNEL_LAYOUT", LAYOUT_DEFAULT) == "t2":
        # (NCORES*128, 8, B): [c*128+p, k, b] -> full[b, c*1024 + k*128 + p]
        nk = COLS_OUT_PC // 128
        full = (
            out_cat.reshape(NCORES, 128, nk, B)
            .transpose(3, 0, 2, 1)
            .reshape(B, NCORES * COLS_OUT_PC)
            .astype(np.float32)
        )
    elif os.environ.get("KERNEL_LAYOUT", LAYOUT_DEFAULT) == "tout":
        # (NCORES*COLS_OUT_PC, B) -> (B, NCORES*COLS_OUT_PC)
        full = (
            out_cat.reshape(NCORES, COLS_OUT_PC, B)
            .transpose(2, 0, 1)
            .reshape(B, NCORES * COLS_OUT_PC)
            .astype(np.float32)
        )
    else:
        # (NCORES*B, COLS_OUT_PC) -> (B, NCORES*COLS_OUT_PC)
        full = np.concatenate(
            [
                out_cat[c * B : (c + 1) * B].astype(np.float32)
                for c in range(NCORES)
            ],
            axis=1,
        )
    return full



# revision 29
# speedup vs baseline: 1.2326x; 1.0057x over previous
"""Trainium2 Bass kernel for nn_BlockDense_89730456748629.

Block-diagonal dense layer + ReLU:
    out[b, g*H+h] = relu( sum_w inputs[b, g*WIN+w] * W[g*WIN+w, g*H+h] )
with G=32 groups, WIN=128, H=256, B=4096.

Sharding: group-parallel over 8 NeuronCores — core c owns groups
[4c, 4c+4). Each core gets the matching 512 input columns of `inputs`
(pre-transposed on host so the contraction dim lies on SBUF partitions)
plus its 4 diagonal W blocks, and produces the matching 1024 output
columns. No cross-core communication.

Default layout "t2" (measured ~7% faster than the earlier "bchunk"):
  - W-block-half [128w,128h] is the stationary matmul operand; batch
    streams through at N=512 into fp32 PSUM banks -> PSUM lands h-major.
  - ReLU fused into the PSUM->SBUF f16 copy, alternating VectorE /
    ScalarE 1:1 (both ~0.6-0.7us per [128,512] tile; neither issues DMAs).
  - Output DRAM layout is partition-major [128, 8, B] per core (host
    decodes): per-partition runs are 16-64KB contiguous, so an 8MB write
    stream needs only ~512 descriptors (vs 4096 with 2KB runs) and the
    SP-ring sequencer issues just 4 dma_starts per iteration.
  - All DMAs ride the SP HWDGE ring; ACT/DVE do pure relu.
  - Reads for two iterations are fetched in one burst (KERNEL_INBURST=2),
    halving HBM read/write direction switches (~1us/iter measured).

Measured rooflines on these cores (f16): writes ~345GB/s, reads
~390-425GB/s, mixed R/W ~310-330GB/s aggregate regardless of ring
split — the kernel is pinned at the mixed-traffic HBM wall (12MB/iter),
with PE (~16us) and relu (~21us) fully hidden.
"""

import os
import time

import numpy as np

G, WIN, H, B = 32, 128, 256, 4096
NCORES = 8
GPC = G // NCORES            # groups per core
COLS_IN_PC = GPC * WIN       # 512 input columns per core
COLS_OUT_PC = GPC * H        # 1024 output columns per core
NB = B // 128                # 32 batch tiles of 128 rows

# dtype config: f32 | f32r | f16 | bf16 for inputs/matmul, f32 | f16 | bf16 out.
# Default f16 end-to-end: measured output error is dominated by the final
# f16 rounding (~5e-4 scale-relative max) while DMA bytes (the bottleneck)
# halve vs f32.
IN_DT = os.environ.get("KERNEL_IN_DT", "f16")
OUT_DT = os.environ.get("KERNEL_OUT_DT", "f16")
# batch tiles per out-DMA chunk (2-byte out: 16 -> 4MB chunks; 4-byte: 8)
CH = int(
    os.environ.get("KERNEL_CH", "16" if OUT_DT in ("f16", "bf16") else "8")
)
VERBOSE = os.environ.get("KERNEL_VERBOSE", "0") == "1"
# single source of truth for the layout default (used by _build and kernel())
LAYOUT_DEFAULT = "t2"

_progs = {}


def _log(msg):
    if VERBOSE:
        print(f"[kernel] {msg}", flush=True)


def _np_dt(tag):
    if tag in ("f32", "f32r"):
        return np.dtype(np.float32)
    if tag == "f16":
        return np.dtype(np.float16)
    if tag == "bf16":
        import ml_dtypes

        return np.dtype(ml_dtypes.bfloat16)
    raise ValueError(tag)


def _mybir_dt(tag):
    from concourse import mybir

    return {
        "f32": mybir.dt.float32,
        "f32r": mybir.dt.float32r,
        "f16": mybir.dt.float16,
        "bf16": mybir.dt.bfloat16,
    }[tag]


def _build(in_tag, out_tag, repeat, loop_n=0):
    """Build the program. `repeat` = static unroll of the whole body;
    `loop_n` > 0 additionally wraps the unrolled body in a hardware
    For_i loop with that trip count (bench-only, for timing)."""
    from concourse import bacc, mybir, tile

    # bench-only ablations: comma-set of {noin,nomm,norelu,noout}
    ablate = set(filter(None, os.environ.get("KERNEL_ABLATE", "").split(",")))
    # relu engine split: dve | act | mix (alternate) | mixN (N dve : 1 act)
    relu_eng = os.environ.get("KERNEL_RELU", "mix")
    psw = int(os.environ.get("KERNEL_PSW", "512"))   # psum tile width (256|512)
    # psum dtype for tout layout: f32 | bf16 | f16 (16-bit doubles DVE rate)
    ps_tag = os.environ.get("KERNEL_PSDT", "f32")
    # bchunk | group | tout | t2. t2 measured ~7% faster than bchunk:
    # W-stationary matmuls, partition-major out layout (64KB runs, 32x
    # fewer descriptors), out-DMAs on the idle SP ring, ACT+DVE pure relu.
    layout = os.environ.get("KERNEL_LAYOUT", LAYOUT_DEFAULT)
    # ring for input DMAs: "sync" = separate ring from out-DMAs (full
    # concurrency, HBM pays read/write turnaround), "act" = same ring as
    # out-DMAs (FIFO phases read bursts vs write bursts), "both" = alternate
    inring = os.environ.get("KERNEL_INRING", "sync")
    outring = os.environ.get("KERNEL_OUTRING", "act")  # act | both
    # phase=1: order in-DMA burst k+1 after the last out-DMA of k so HBM
    # sees alternating read/write bursts instead of mixed traffic
    phase = os.environ.get("KERNEL_PHASE", "0") == "1"

    in_dt = _mybir_dt(in_tag)
    out_dt = _mybir_dt(out_tag)

    nc = bacc.Bacc(
        "TRN2", target_bir_lowering=False, debug=False, num_devices=NCORES
    )
    xT = nc.declare_dram_parameter("xT", [COLS_IN_PC, B], in_dt, isOutput=False)
    Wb = nc.declare_dram_parameter("Wb", [WIN, COLS_OUT_PC], in_dt, isOutput=False)
    if layout == "t2":
        # partition-major transposed output: out[p, k, b] = column k*128+p,
        # batch b. Per-partition DRAM runs are 8*B*2 = 64KB contiguous ->
        # 128 descriptors per 8MB. Host decodes.
        out = nc.declare_dram_parameter(
            "out", [128, GPC * (H // 128), B], out_dt, isOutput=True
        )
        out_v = None
    elif layout == "tout":
        # transposed output: row = output column (g*H + h), col = batch.
        # Host re-transposes; device DMA gets 8KB contiguous runs/partition.
        out = nc.declare_dram_parameter("out", [COLS_OUT_PC, B], out_dt, isOutput=True)
        out_v = None
    else:
        out = nc.declare_dram_parameter("out", [B, COLS_OUT_PC], out_dt, isOutput=True)
        out_v = out.rearrange("(nb p) w -> nb p w", p=128)  # (NB, 128, COLS_OUT_PC)

    in_sz = 2 if in_tag in ("f16", "bf16") else 4
    out_sz = 2 if out_tag in ("f16", "bf16") else 4
    if layout == "bchunk":
        # deep prefetch wins: 4 resident group tiles + 8 in flight ahead
        xbufs = 12 if in_sz == 2 else 6
        if out_sz == 2:
            obufs = 3 if CH >= 16 else 5
        else:
            obufs = 2
    elif layout == "tout":
        # xt and ob tiles are both 8KB/partition at f16
        xbufs = 8 if in_sz == 2 else 4
        obufs = 4 if out_sz == 2 else 2
    elif layout == "t2":
        ogrp = int(os.environ.get("KERNEL_OGRP", "2"))
        inb = int(os.environ.get("KERNEL_INBURST", "2"))
        if os.environ.get("KERNEL_IN1", "0") == "1":
            # fused xt tiles are 4x bigger (32KB/partition at f16)
            xbufs = inb + 1
        else:
            # 4*inburst resident xt tiles + prefetch margin (8KB/part each)
            xbufs = (6 if in_sz == 2 else 4) + 4 * (inb - 1)
        # ob tile = ogrp*8KB/partition; keep ~2 iterations in flight
        obufs = max(2, min(16 // ogrp, 6 if inb == 1 else 4))
    else:
        xbufs, obufs = 2, 4
    xbufs = int(os.environ.get("KERNEL_XBUFS", xbufs))
    obufs = int(os.environ.get("KERNEL_OBUFS", obufs))

    with tile.TileContext(nc) as tc:
        with (
            tc.tile_pool(name="w", bufs=1) as wpool,
            tc.tile_pool(name="x", bufs=xbufs) as xpool,
            tc.tile_pool(name="o", bufs=obufs) as opool,
            tc.tile_pool(name="ps", bufs=8, space="PSUM") as pspool,
        ):
            wt = wpool.tile([WIN, COLS_OUT_PC], in_dt)
            nc.sync.dma_start(wt[:], Wb[:, :])

            relu_ct = [0]
            # mixN: N dve ops per 1 act op (cost model: roughly equal rates)
            mix_n = 1
            if relu_eng.startswith("mix") and len(relu_eng) > 3:
                mix_n = int(relu_eng[3:])
                assert mix_n <= 9, f"suspicious mix ratio {mix_n}"

            def relu(dst, src):
                pick = relu_eng
                if pick.startswith("mix"):
                    pick = "dve" if relu_ct[0] % (mix_n + 1) < mix_n else "act"
                relu_ct[0] += 1
                if pick == "dve":
                    nc.vector.tensor_scalar_max(dst, src, 0.0)
                else:
                    nc.scalar.activation(
                        dst, src, mybir.ActivationFunctionType.Relu
                    )

            mm_per_ps = psw // H  # matmuls per psum tile (1 or 2)

            def body_group():
                """Group-outer: xt = one group row over all B; out-DMA
                writes H-wide column strips (512B runs at f16)."""
                for _rep in range(repeat):
                    for g in range(GPC):
                        xt = xpool.tile([WIN, B], in_dt)
                        if "noin" not in ablate:
                            nc.sync.dma_start(
                                xt[:], xT[g * WIN : (g + 1) * WIN, :]
                            )
                        for c in range(NB // CH):
                            ob = opool.tile([128, CH * H], out_dt)
                            for j2 in range(CH // mm_per_ps):
                                ps = pspool.tile([128, psw], mybir.dt.float32)
                                for h in range(mm_per_ps):
                                    bt = c * CH + j2 * mm_per_ps + h
                                    if "nomm" not in ablate:
                                        nc.tensor.matmul(
                                            ps[:, h * H : (h + 1) * H],
                                            xt[:, bt * 128 : (bt + 1) * 128],
                                            wt[:, g * H : (g + 1) * H],
                                            start=True,
                                            stop=True,
                                        )
                                if "norelu" not in ablate:
                                    relu(
                                        ob[:, j2 * psw : (j2 + 1) * psw],
                                        ps[:],
                                    )
                            if "noout" not in ablate:
                                dv = out_v[
                                    c * CH : (c + 1) * CH, :, g * H : (g + 1) * H
                                ].transpose([1, 0, 2])
                                # out-DMAs ride the ACT HWDGE ring so they
                                # overlap the input DMAs on the SP ring
                                # (FIFO per ring)
                                ob3 = ob[:].rearrange("p (j h) -> p j h", h=H)
                                nc.scalar.dma_start(dv, ob3)

            def body_bchunk():
                """B-chunk-outer: all 4 group tiles resident; out-DMA
                writes full COLS_OUT_PC-wide rows (2KB runs at f16)."""
                from concourse.tile import add_dep_helper

                in1 = os.environ.get("KERNEL_IN1", "0") == "1"
                prev_out = [None]
                for _rep in range(repeat):
                    if in1:
                        # one fused 4MB input DMA: xT shard rows (g, p) -> p g b
                        xt_all = xpool.tile([WIN, GPC, B], in_dt, tag="xt")
                        if "noin" not in ablate:
                            nc.sync.dma_start(
                                xt_all[:],
                                xT.rearrange("(g p) b -> p g b", p=WIN),
                            )
                        xts = [xt_all[:, g, :] for g in range(GPC)]
                    else:
                        xts = []
                    for g in range(GPC if not in1 else 0):
                        if inring == "both":
                            in_eng = nc.sync if g % 2 == 0 else nc.scalar
                        elif inring == "gpsimd":
                            in_eng = nc.gpsimd
                        else:
                            in_eng = nc.scalar if inring == "act" else nc.sync
                        xt = xpool.tile([WIN, B], in_dt, tag="xt")
                        if "noin" not in ablate:
                            if inring == "sync2":
                                # split each group read into two halves for
                                # more descriptors in flight
                                hb = B // 2
                                for s in range(2):
                                    di = nc.sync.dma_start(
                                        xt[:, s * hb : (s + 1) * hb],
                                        xT[
                                            g * WIN : (g + 1) * WIN,
                                            s * hb : (s + 1) * hb,
                                        ],
                                    )
                            else:
                                di = in_eng.dma_start(
                                    xt[:], xT[g * WIN : (g + 1) * WIN, :]
                                )
                            if phase and prev_out[0] is not None:
                                add_dep_helper(
                                    prev_out[0].ins,
                                    di.ins,
                                    True,
                                    "phase reads after writes",
                                )
                        xts.append(xt)
                    for c in range(NB // CH):
                        ob = opool.tile([128, CH, COLS_OUT_PC], out_dt)
                        if "norelu" in ablate and "noout" not in ablate:
                            # mark ob written so Tile allocates it (bench only)
                            nc.gpsimd.memset(ob[:, 0, 0:128], 0)
                        for g in range(GPC):
                            for j2 in range(CH // mm_per_ps):
                                ps = pspool.tile([128, psw], mybir.dt.float32)
                                for h in range(mm_per_ps):
                                    bt = c * CH + j2 * mm_per_ps + h
                                    if "nomm" not in ablate:
                                        nc.tensor.matmul(
                                            ps[:, h * H : (h + 1) * H],
                                            xts[g][:, bt * 128 : (bt + 1) * 128],
                                            wt[:, g * H : (g + 1) * H],
                                            start=True,
                                            stop=True,
                                        )
                                if "norelu" not in ablate:
                                    # psum [128, (j, h)] -> ob rows j2*m+h,
                                    # group-g column strip
                                    dst = ob[
                                        :,
                                        j2 * mm_per_ps : (j2 + 1) * mm_per_ps,
                                        g * H : (g + 1) * H,
                                    ]
                                    src = ps[:].rearrange(
                                        "p (j h) -> p j h", h=H
                                    )
                                    relu(dst, src)
                        if "noout" not in ablate:
                            dv = out_v[c * CH : (c + 1) * CH, :, :].transpose(
                                [1, 0, 2]
                            )
                            if outring == "both":
                                out_eng = nc.scalar if c % 2 == 0 else nc.sync
                            elif outring == "sync":
                                out_eng = nc.sync
                            else:
                                out_eng = nc.scalar
                            do = out_eng.dma_start(dv, ob[:])
                            prev_out[0] = do

            def body_tout():
                """Transposed-output: W-block-half [128w,128h] is the
                stationary matmul operand, batch streams through at N=psw.
                PSUM lands [128h, b] so the out-DMA writes outT rows with
                8KB contiguous runs per partition. One 1MB out-DMA per
                (group, half), optionally striped across both HWDGE rings
                (KERNEL_OSPLIT=2)."""
                ps_dt = {
                    "f32": mybir.dt.float32,
                    "bf16": mybir.dt.bfloat16,
                    "f16": mybir.dt.float16,
                }[ps_tag]
                osplit = int(os.environ.get("KERNEL_OSPLIT", "1"))
                in1 = os.environ.get("KERNEL_IN1", "0") == "1"
                nhalf = H // 128  # 2 halves per group
                for _rep in range(repeat):
                    if in1:
                        # one fused 4MB input DMA: (g p) b -> p g b
                        xt_all = xpool.tile([WIN, GPC, B], in_dt, tag="xt")
                        if "noin" not in ablate:
                            nc.sync.dma_start(
                                xt_all[:],
                                xT.rearrange("(g p) b -> p g b", p=WIN),
                            )
                        xts = [xt_all[:, g, :] for g in range(GPC)]
                    else:
                        in_eng = nc.gpsimd if inring == "gpsimd" else nc.sync
                        xts = []
                        for g in range(GPC):
                            xt = xpool.tile([WIN, B], in_dt, tag="xt")
                            if "noin" not in ablate:
                                in_eng.dma_start(
                                    xt[:], xT[g * WIN : (g + 1) * WIN, :]
                                )
                            xts.append(xt)
                    for g in range(GPC):
                        for q in range(nhalf):
                            ob = opool.tile([128, B], out_dt, tag="ob")
                            if "norelu" in ablate and "noout" not in ablate:
                                nc.gpsimd.memset(ob[:, 0:128], 0)
                            wcol = g * H + q * 128
                            for j in range(B // psw):
                                ps = pspool.tile([128, psw], ps_dt)
                                if "nomm" not in ablate:
                                    nc.tensor.matmul(
                                        ps[:],
                                        wt[:, wcol : wcol + 128],
                                        xts[g][:, j * psw : (j + 1) * psw],
                                        start=True,
                                        stop=True,
                                    )
                                if "norelu" not in ablate:
                                    relu(ob[:, j * psw : (j + 1) * psw], ps[:])
                            if "noout" not in ablate:
                                r0 = (g * nhalf + q) * 128
                                k = g * nhalf + q
                                if osplit == 2:
                                    hb = B // 2
                                    for s in range(2):
                                        eng = nc.scalar if s == 0 else nc.sync
                                        eng.dma_start(
                                            out[
                                                r0 : r0 + 128,
                                                s * hb : (s + 1) * hb,
                                            ],
                                            ob[:, s * hb : (s + 1) * hb],
                                        )
                                else:
                                    if outring == "both":
                                        out_eng = (
                                            nc.scalar if k % 2 == 0 else nc.sync
                                        )
                                    elif outring == "sync":
                                        out_eng = nc.sync
                                    else:
                                        out_eng = nc.scalar
                                    out_eng.dma_start(
                                        out[r0 : r0 + 128, :], ob[:]
                                    )

            def body_t2():
                """Like tout but ob stages OGRP (group,half) strips in one
                [128, OGRP, B] tile and writes them with ONE dma_start into
                the [128, 8, B] DRAM layout (64KB runs/partition). Out-DMAs
                ride outring (default SP); ACT+DVE do pure relu."""
                ps_dt = {
                    "f32": mybir.dt.float32,
                    "bf16": mybir.dt.bfloat16,
                    "f16": mybir.dt.float16,
                }[ps_tag]
                ogrp = int(os.environ.get("KERNEL_OGRP", "2"))
                # read-burst width in iterations: fetching 2 iterations of x
                # per burst halves HBM read/write direction switches
                inburst = int(os.environ.get("KERNEL_INBURST", "2"))
                # in1=1: one fused 4MB read per iteration instead of 4x1MB
                # (large DMAs sustain closer to peak: 341GB/s @1MB vs 425 @16MB)
                in1 = os.environ.get("KERNEL_IN1", "0") == "1"
                nhalf = H // 128
                nk = GPC * nhalf  # 8 column strips of 128
                in_eng = nc.gpsimd if inring == "gpsimd" else nc.sync
                xts_pending = {}
                for _rep in range(repeat):
                    if _rep % inburst == 0:
                        for r2 in range(_rep, min(_rep + inburst, repeat)):
                            if in1:
                                xt_all = xpool.tile(
                                    [WIN, GPC, B], in_dt, tag="xt"
                                )
                                if "noin" not in ablate:
                                    in_eng.dma_start(
                                        xt_all[:],
                                        xT.rearrange("(g p) b -> p g b", p=WIN),
                                    )
                                xts_pending[(r2,)] = xt_all
                            else:
                                for g in range(GPC):
                                    xt = xpool.tile([WIN, B], in_dt, tag="xt")
                                    if "noin" not in ablate:
                                        in_eng.dma_start(
                                            xt[:],
                                            xT[g * WIN : (g + 1) * WIN, :],
                                        )
                                    xts_pending[(r2, g)] = xt
                    if in1:
                        xt_all = xts_pending.pop((_rep,))
                        xts = [xt_all[:, g, :] for g in range(GPC)]
                    else:
                        xts = [xts_pending.pop((_rep, g)) for g in range(GPC)]
                    for k0 in range(0, nk, ogrp):
                        ob = opool.tile([128, ogrp, B], out_dt, tag="ob")
                        if "norelu" in ablate and "noout" not in ablate:
                            nc.gpsimd.memset(ob[:, 0, 0:128], 0)
                        for kk in range(ogrp):
                            k = k0 + kk
                            g, q = divmod(k, nhalf)
                            wcol = g * H + q * 128
                            for j in range(B // psw):
                                ps = pspool.tile([128, psw], ps_dt)
                                if "nomm" not in ablate:
                                    nc.tensor.matmul(
                                        ps[:],
                                        wt[:, wcol : wcol + 128],
                                        xts[g][:, j * psw : (j + 1) * psw],
                                        start=True,
                                        stop=True,
                                    )
                                if "norelu" not in ablate:
                                    relu(
                                        ob[:, kk, j * psw : (j + 1) * psw],
                                        ps[:],
                                    )
                        if "noout" not in ablate:
                            if outring == "both":
                                out_eng = (
                                    nc.sync if (k0 // ogrp) % 2 == 0 else nc.scalar
                                )
                            elif outring == "act":
                                out_eng = nc.scalar
                            else:
                                out_eng = nc.sync
                            out_eng.dma_start(
                                out[:, k0 : k0 + ogrp, :], ob[:]
                            )

            body = {
                "bchunk": body_bchunk,
                "group": body_group,
                "tout": body_tout,
                "t2": body_t2,
            }[layout]

            if loop_n > 0:
                sreset = os.environ.get("KERNEL_SRESET", "0") == "1"
                with tc.For_i(0, loop_n, 1, staggered_reset=sreset):
                    body()
            else:
                body()
    nc.compile()
    return nc


def _make_runner(nc):
    """Cached jitted shard_map runner over 8 cores (modeled on
    concourse.bass2jax.run_bass_via_pjrt, but reusable across calls:
    the jitted fn and on-device zero output buffers are kept)."""
    import jax

    try:  # soften repeat first-call compiles across processes
        jax.config.update("jax_compilation_cache_dir", "/tmp/jax_bass_cache")
        jax.config.update("jax_persistent_cache_min_compile_time_secs", 1.0)
    except Exception:
        pass
    from jax.experimental.shard_map import shard_map
    from jax.sharding import Mesh, NamedSharding, PartitionSpec

    from concourse import mybir
    from concourse.bass2jax import (
        _bass_exec_p,
        install_neuronx_cc_hook,
        partition_id_tensor,
    )

    install_neuronx_cc_hook()

    partition_name = (
        nc.partition_id_tensor.name if nc.partition_id_tensor else None
    )
    in_names, out_names, out_avals = [], [], []
    for alloc in nc.m.functions[0].allocations:
        if not isinstance(alloc, mybir.MemoryLocationSet):
            continue
        name = alloc.memorylocations[0].name
        if alloc.kind == "ExternalInput":
            if name != partition_name:
                in_names.append(name)
        elif alloc.kind == "ExternalOutput":
            out_names.append(name)
            out_avals.append(
                jax.core.ShapedArray(
                    tuple(alloc.tensor_shape), mybir.dt.np(alloc.dtype)
                )
            )
    n_params = len(in_names)
    all_names = in_names + out_names
    if partition_name is not None:
        all_names = all_names + [partition_name]

    def _body(*args):
        operands = list(args)
        if partition_name is not None:
            operands.append(partition_id_tensor())
        outs = _bass_exec_p.bind(
            *operands,
            out_avals=tuple(out_avals),
            in_names=tuple(all_names),
            out_names=tuple(out_names),
            lowering_input_output_aliases=(),
            sim_require_finite=True,
            sim_require_nnan=True,
            nc=nc,
        )
        return tuple(outs)

    devices = jax.devices()[:NCORES]
    mesh = Mesh(np.asarray(devices), ("core",))
    nout = len(out_names)
    fn = jax.jit(
        shard_map(
            _body,
            mesh=mesh,
            in_specs=(PartitionSpec("core"),) * (n_params + nout),
            out_specs=(PartitionSpec("core"),) * nout,
            check_rep=False,
        ),
        keep_unused=True,
    )
    sharding = NamedSharding(mesh, PartitionSpec("core"))
    zeros = [
        jax.device_put(
            np.zeros((NCORES * a.shape[0], *a.shape[1:]), a.dtype), sharding
        )
        for a in out_avals
    ]
    return {
        "fn": fn,
        "in_names": in_names,
        "out_names": out_names,
        "out_avals": out_avals,
        "sharding": sharding,
        "zeros": zeros,
    }


def get_prog(repeat=1, loop_n=0):
    """Build (or fetch cached) compiled program + runner for the current
    dtype config and the given repeat-unroll / hw-loop factors."""
    key = (IN_DT, OUT_DT, repeat, loop_n)
    if key not in _progs:
        t0 = time.time()
        nc = _build(IN_DT, OUT_DT, repeat, loop_n)
        t1 = time.time()
        runner = _make_runner(nc)
        t2 = time.time()
        _log(
            f"built prog {key}: bass build+compile {t1 - t0:.1f}s, "
            f"runner setup {t2 - t1:.1f}s"
        )
        runner["nc"] = nc
        _progs[key] = runner
    return _progs[key]


def shard_inputs(inputs, W):
    """Host-side sharding: transpose x, extract diagonal W blocks, split
    per core, concat along axis 0 for shard_map consumption."""
    in_np = _np_dt(IN_DT)
    x = np.asarray(inputs, dtype=np.float32)
    Wf = np.asarray(W, dtype=np.float32)

    xT = np.ascontiguousarray(x.T)  # (G*WIN, B): row g*WIN+w = input col
    Wd = Wf.reshape(G, WIN, G, H)[np.arange(G), :, np.arange(G), :]  # (G,WIN,H)

    # concat over cores along axis 0 (shard_map splits axis 0 across mesh)
    xT_cat = xT.astype(in_np)  # already (NCORES*COLS_IN_PC, B) in core order
    Wb_cat = np.ascontiguousarray(
        Wd.reshape(NCORES, GPC, WIN, H)
        .transpose(0, 2, 1, 3)
        .reshape(NCORES * WIN, COLS_OUT_PC)
    ).astype(in_np)
    return {"xT": xT_cat, "Wb": Wb_cat}


def place_inputs(prog, cat_inputs):
    """device_put the sharded inputs once; reusable across run_prog calls."""
    import jax

    return [
        jax.device_put(cat_inputs[name], prog["sharding"])
        for name in prog["in_names"]
    ]


def run_prog(prog, cat_inputs=None, placed=None):
    """Run the program on 8 cores; returns output arrays (on device)."""
    import jax

    if placed is None:
        placed = place_inputs(prog, cat_inputs)
    outs = prog["fn"](*placed, *prog["zeros"])
    jax.block_until_ready(outs)
    return outs


def kernel(inputs, W):
    prog = get_prog(repeat=1)
    cat = shard_inputs(inputs, W)
    outs = run_prog(prog, cat)
    out_cat = np.asarray(outs[prog["out_names"].index("out")])
    if os.environ.get("KERNEL_LAYOUT", LAYOUT_DEFAULT) == "t2":
        # (NCORES*128, 8, B): [c*128+p, k, b] -> full[b, c*1024 + k*128 + p]
        nk = COLS_OUT_PC // 128
        full = (
            out_cat.reshape(NCORES, 128, nk, B)
            .transpose(3, 0, 2, 1)
            .reshape(B, NCORES * COLS_OUT_PC)
            .astype(np.float32)
        )
    elif os.environ.get("KERNEL_LAYOUT", LAYOUT_DEFAULT) == "tout":
        # (NCORES*COLS_OUT_PC, B) -> (B, NCORES*COLS_OUT_PC)
        full = (
            out_cat.reshape(NCORES, COLS_OUT_PC, B)
            .transpose(2, 0, 1)
            .reshape(B, NCORES * COLS_OUT_PC)
            .astype(np.float32)
        )
    else:
        # (NCORES*B, COLS_OUT_PC) -> (B, NCORES*COLS_OUT_PC)
        full = np.concatenate(
            [
                out_cat[c * B : (c + 1) * B].astype(np.float32)
                for c in range(NCORES)
            ],
            axis=1,
        )
    return full

